# revision 2
# baseline (speedup 1.0000x reference)
"""Trainium2 Bass kernel for nn_Cat_20916490732013.

Computes out = log_softmax(logits, axis=-1) where
    L = start_emb @ proj                  # [n, f]
    R = output_emb @ proj                 # [c, f]
    logits = log(exp(L) @ exp(R).T) / temp

Sharding: column-parallel over the class axis (8 cores, 4096 classes each).
start_emb and proj are replicated; output_emb and the output are sharded.

Key algebraic identity used to avoid a second pass over the [n, c] logits:
    logsumexp_j(logits_i) = log( sum_j sum_f exp(L_if) exp(R_jf) )
                          = log( sum_f exp(L_if) * V_f ),   V_f = sum_j exp(R_jf)
so each core computes a per-row partial Z_i^(c) = sum_f exp(L_if) * v_f^(c)
(v from its own class shard) and a single 16 KiB AllReduce(add) of Z yields the
global normalizer. The output tile epilogue is then one scalar-engine op:
    out_ij = Ln(S_ij * (1/Z_i)),  S = exp(L) @ exp(R).T
(The input distribution keeps |L|,|R| < ~1, so unstabilized exp is exact-safe;
the per-row max terms cancel identically in log-softmax.)
"""

import sys

for _p in ("/opt/trn_rl_repo",):
    if _p not in sys.path:
        sys.path.insert(0, _p)

from contextlib import ExitStack

import numpy as np

N_CORES = 8
N_STARTS = 4096
N_CLASSES = 32768
EMB = 128
FEAT = 256
SHARD = N_CLASSES // N_CORES  # 4096

_CACHE = {}


def build_bass(n=N_STARTS, shard=SHARD, feat=FEAT, emb=EMB, n_cores=N_CORES):
    """Build + compile the per-core Bass program (SPMD: same program on all cores)."""
    import concourse.tile as tile
    from concourse import bacc, mybir
    from concourse.masks import make_identity

    f32 = mybir.dt.float32
    P = 128
    assert emb == P and feat % P == 0 and n % P == 0 and shard % P == 0
    FB = feat // P            # feature blocks on the partition axis (2)
    NT = n // P               # output row tiles (32)
    JT = shard // P           # class tiles per core (32)
    NCH = 512                 # matmul free-dim per instruction (fp32 max)
    JW = min(4 * NCH, shard)  # psum group width for the main matmul (2048)
    JH = shard // JW          # psum groups per row tile

    Exp = mybir.ActivationFunctionType.Exp
    Ln = mybir.ActivationFunctionType.Ln
    X = mybir.AxisListType.X

    nc = bacc.Bacc("TRN2", target_bir_lowering=False, debug=False,
                   num_devices=n_cores)
    se_d = nc.dram_tensor("se", [n, emb], f32, kind="ExternalInput").ap()
    oe_d = nc.dram_tensor("oe", [shard, emb], f32, kind="ExternalInput").ap()
    pj_d = nc.dram_tensor("proj", [emb, feat], f32, kind="ExternalInput").ap()
    out_d = nc.dram_tensor("out", [n, shard], f32, kind="ExternalOutput").ap()

    with ExitStack() as ctx:
        tc = ctx.enter_context(tile.TileContext(nc))
        const = ctx.enter_context(tc.tile_pool(name="const", bufs=1))
        big = ctx.enter_context(tc.tile_pool(name="big", bufs=1))
        obp = ctx.enter_context(tc.tile_pool(name="obp", bufs=3))
        dram = ctx.enter_context(tc.tile_pool(name="dram", bufs=1, space="DRAM"))

        ident = const.tile([P, P], f32, name="ident")
        make_identity(nc, ident)
        proj_sb = const.tile([P, feat], f32, name="proj_sb")
        nc.sync.dma_start(out=proj_sb, in_=pj_d)

        # Inputs in natural layout: partition = row-within-tile.
        se_sb = big.tile([P, NT, emb], f32, name="se_sb")
        oe_sb = big.tile([P, JT, emb], f32, name="oe_sb")
        se_r = se_d.rearrange("(t p) k -> p t k", p=P)
        oe_r = oe_d.rearrange("(t p) k -> p t k", p=P)
        # Chunked loads so transposes can start before the full tensor lands.
        ldc = 8
        for g in range(0, NT, ldc):
            ge = min(g + ldc, NT)
            nc.sync.dma_start(out=se_sb[:, g:ge, :], in_=se_r[:, g:ge, :])
        for g in range(0, JT, ldc):
            ge = min(g + ldc, JT)
            nc.sync.dma_start(out=oe_sb[:, g:ge, :], in_=oe_r[:, g:ge, :])

        # Transposed inputs: [k, n] / [k, j] — contraction dim on partitions.
        seT = big.tile([P, n], f32, name="seT")
        oeT = big.tile([P, shard], f32, name="oeT")
        # exp of projections, transposed: [f, n] / [f, j], split into FB blocks.
        elT = [big.tile([P, n], f32, name=f"elT{fb}") for fb in range(FB)]
        erT = [big.tile([P, shard], f32, name=f"erT{fb}") for fb in range(FB)]
        vown = const.tile([P, FB], f32, name="vown")
        z_sb = const.tile([P, NT], f32, name="z_sb")
        Z_sb = const.tile([P, NT], f32, name="Z_sb")
        invz = const.tile([P, NT], f32, name="invz")

        # ---- Phase 1: transpose inputs, project, exp, partial normalizer ----
        with tc.tile_pool(name="ps_tr", bufs=3, space="PSUM") as ps_tr, \
             tc.tile_pool(name="ps_pj", bufs=3, space="PSUM") as ps_pj, \
             tc.tile_pool(name="ps_z", bufs=2, space="PSUM") as ps_z:
            for src, dstT, tcnt in ((se_sb, seT, NT), (oe_sb, oeT, JT)):
                for t in range(tcnt):
                    pst = ps_tr.tile([P, P], f32, name="pst", tag="pst")
                    nc.tensor.transpose(pst, src[:, t, :], ident)
                    nc.vector.tensor_copy(out=dstT[:, t * P:(t + 1) * P], in_=pst)

            # LT/RT = projT-block @ (SE.T / OE.T); exp applied on PSUM->SBUF.
            for fb in range(FB):
                pw = proj_sb[:, fb * P:(fb + 1) * P]
                for srcT, dst, width in ((seT, elT[fb], n), (oeT, erT[fb], shard)):
                    for c0 in range(0, width, NCH):
                        w = min(NCH, width - c0)
                        psl = ps_pj.tile([P, NCH], f32, name="psl", tag="psl")
                        nc.tensor.matmul(psl[:, :w], pw, srcT[:, c0:c0 + w],
                                         start=True, stop=True)
                        nc.scalar.activation(out=dst[:, c0:c0 + w], in_=psl[:, :w],
                                             func=Exp)

            # v_f = sum over this core's classes of exp(R_jf)
            for fb in range(FB):
                nc.vector.reduce_sum(out=vown[:, fb:fb + 1], in_=erT[fb], axis=X)

            # Per-row partial normalizer z_i = sum_f exp(L_if) * v_f  -> [n]
            for t in range(NT):
                psz = ps_z.tile([P, 1], f32, name="psz", tag="psz")
                for fb in range(FB):
                    nc.tensor.matmul(psz, elT[fb][:, t * P:(t + 1) * P],
                                     vown[:, fb:fb + 1],
                                     start=(fb == 0), stop=(fb == FB - 1))
                nc.scalar.copy(out=z_sb[:, t:t + 1], in_=psz)

        # ---- AllReduce the per-row partial normalizers (16 KiB) ----
        cc_in = dram.tile([P, NT], f32, name="cc_in")
        cc_out = dram.tile([P, NT], f32, name="cc_out")
        nc.sync.dma_start(out=cc_in, in_=z_sb)
        nc.gpsimd.collective_compute(
            "AllReduce", mybir.AluOpType.add,
            replica_groups=[list(range(n_cores))],
            ins=[cc_in.opt()], outs=[cc_out.opt()],
        )
        nc.sync.dma_start(out=Z_sb, in_=cc_out)
        nc.vector.reciprocal(out=invz, in_=Z_sb)

        # ---- Phase 2: S = exp(L) @ exp(R).T, out = Ln(S * invz), store ----
        with tc.tile_pool(name="ps_mm", bufs=2, space="PSUM") as ps_mm:
            for t in range(NT):
                ob = obp.tile([P, shard], f32, name="ob", tag="ob")
                for jh in range(JH):
                    ps = ps_mm.tile([P, JW], f32, name="ps", tag="ps")
                    for fb in range(FB):
                        lw = elT[fb][:, t * P:(t + 1) * P]
                        for c0 in range(0, JW, NCH):
                            nc.tensor.matmul(
                                ps[:, c0:c0 + NCH], lw,
                                erT[fb][:, jh * JW + c0: jh * JW + c0 + NCH],
                                start=(fb == 0), stop=(fb == FB - 1))
                    nc.scalar.activation(out=ob[:, jh * JW:(jh + 1) * JW],
                                         in_=ps, func=Ln,
                                         scale=invz[:, t:t + 1])
                nc.sync.dma_start(out=out_d[t * P:(t + 1) * P, :], in_=ob)

    nc.compile()
    return nc


def _get_nc():
    if "nc" not in _CACHE:
        _CACHE["nc"] = build_bass()
    return _CACHE["nc"]


def _numpy_fallback(start_emb, output_emb, proj, temp):
    L = start_emb.astype(np.float64) @ proj.astype(np.float64)
    R = output_emb.astype(np.float64) @ proj.astype(np.float64)
    mL = L.max(-1, keepdims=True)
    mR = R.max(-1, keepdims=True)
    S = np.exp(L - mL) @ np.exp(R - mR).T
    logits = (np.log(S) + mL + mR.T) / float(temp)
    m = logits.max(-1, keepdims=True)
    out = logits - m - np.log(np.exp(logits - m).sum(-1, keepdims=True))
    return out.astype(np.float32)


def run_on_hw(start_emb, output_emb, proj, trace=False, **trace_kwargs):
    from concourse.bass_utils import run_bass_kernel_spmd

    nc = _get_nc()
    se = np.ascontiguousarray(start_emb, dtype=np.float32)
    oe = np.ascontiguousarray(output_emb, dtype=np.float32)
    pj = np.ascontiguousarray(proj, dtype=np.float32)
    in_maps = [
        {"se": se, "oe": oe[c * SHARD:(c + 1) * SHARD], "proj": pj}
        for c in range(N_CORES)
    ]
    res = run_bass_kernel_spmd(nc, in_maps, core_ids=list(range(N_CORES)),
                               trace=trace, **trace_kwargs)
    out = np.concatenate([res.results[c]["out"] for c in range(N_CORES)], axis=1)
    return out, res


def kernel(start_emb, output_emb, proj, temp):
    t = float(np.asarray(temp).reshape(-1)[0])
    if t != 1.0:
        return _numpy_fallback(np.asarray(start_emb), np.asarray(output_emb),
                               np.asarray(proj), t)
    out, _ = run_on_hw(start_emb, output_emb, proj, trace=False)
    return out


# revision 6
# speedup vs baseline: 1.9753x; 1.9753x over previous
"""Trainium2 Bass kernel for nn_Cat_20916490732013.

Computes out = log_softmax(logits, axis=-1) where
    L = start_emb @ proj                  # [n, f]
    R = output_emb @ proj                 # [c, f]
    logits = log(exp(L) @ exp(R).T) / temp

Sharding: column-parallel over the class axis (8 cores, 4096 classes each).
start_emb and proj are replicated; output_emb and the output are sharded.

Key algebraic identity used to avoid a second pass over the [n, c] logits:
    logsumexp_j(logits_i) = log( sum_j sum_f exp(L_if) exp(R_jf) )
                          = log( sum_f exp(L_if) * V_f ),   V_f = sum_j exp(R_jf)
so each core computes a per-row partial Z_i^(c) = sum_f exp(L_if) * v_f^(c)
(v from its own class shard) and a single 16 KiB AllReduce(add) of Z yields the
global normalizer. The output tile epilogue is then one scalar-engine op:
    out_ij = Ln(S_ij * (1/Z_i)),  S = exp(L) @ exp(R).T
(The input distribution keeps |L|,|R| < ~1, so unstabilized exp is exact-safe;
the per-row max terms cancel identically in log-softmax.)
"""

import sys

for _p in ("/opt/trn_rl_repo",):
    if _p not in sys.path:
        sys.path.insert(0, _p)

from contextlib import ExitStack

import numpy as np

N_CORES = 8
N_STARTS = 4096
N_CLASSES = 32768
EMB = 128
FEAT = 256
SHARD = N_CLASSES // N_CORES  # 4096

_CACHE = {}


def build_bass(n=N_STARTS, shard=SHARD, feat=FEAT, emb=EMB, n_cores=N_CORES,
               use_bf16=True):
    """Build + compile the per-core Bass program (SPMD: same program on all cores)."""
    import concourse.tile as tile
    from concourse import bacc, mybir
    from concourse.masks import make_identity

    f32 = mybir.dt.float32
    bf16 = mybir.dt.bfloat16
    mm_dt = bf16 if use_bf16 else f32
    P = 128
    assert emb == P and feat % P == 0 and n % P == 0 and shard % P == 0
    FB = feat // P            # feature blocks on the partition axis (2)
    NT = n // P               # output row tiles (32)
    JT = shard // P           # class tiles per core (32)
    NCH = 512                 # matmul free-dim per instruction (fp32 max)
    JW = min(4 * NCH, shard)  # psum group width for the main matmul (2048)
    JH = shard // JW          # psum groups per row tile

    Exp = mybir.ActivationFunctionType.Exp
    Ln = mybir.ActivationFunctionType.Ln
    X = mybir.AxisListType.X

    nc = bacc.Bacc("TRN2", target_bir_lowering=False, debug=False,
                   num_devices=n_cores)
    se_d = nc.dram_tensor("se", [n, emb], f32, kind="ExternalInput").ap()
    oe_d = nc.dram_tensor("oe", [shard, emb], f32, kind="ExternalInput").ap()
    pj_d = nc.dram_tensor("proj", [emb, feat], f32, kind="ExternalInput").ap()
    out_d = nc.dram_tensor("out", [n, shard], f32, kind="ExternalOutput").ap()

    with ExitStack() as ctx:
        tc = ctx.enter_context(tile.TileContext(nc))
        const = ctx.enter_context(tc.tile_pool(name="const", bufs=1))
        big = ctx.enter_context(tc.tile_pool(name="big", bufs=1))
        obp = ctx.enter_context(tc.tile_pool(name="obp", bufs=3))
        dram = ctx.enter_context(tc.tile_pool(name="dram", bufs=1, space="DRAM"))

        ident = const.tile([P, P], f32, name="ident")
        make_identity(nc, ident)
        proj_sb = const.tile([P, feat], f32, name="proj_sb")
        nc.sync.dma_start(out=proj_sb, in_=pj_d)

        # Inputs in natural layout: partition = row-within-tile.
        se_sb = big.tile([P, NT, emb], f32, name="se_sb")
        oe_sb = big.tile([P, JT, emb], f32, name="oe_sb")
        se_r = se_d.rearrange("(t p) k -> p t k", p=P)
        oe_r = oe_d.rearrange("(t p) k -> p t k", p=P)
        # Chunked loads so transposes can start before the full tensor lands.
        ldc = 8
        for g in range(0, NT, ldc):
            ge = min(g + ldc, NT)
            nc.sync.dma_start(out=se_sb[:, g:ge, :], in_=se_r[:, g:ge, :])
        for g in range(0, JT, ldc):
            ge = min(g + ldc, JT)
            nc.sync.dma_start(out=oe_sb[:, g:ge, :], in_=oe_r[:, g:ge, :])

        # Transposed inputs: [k, n] / [k, j] — contraction dim on partitions.
        seT = big.tile([P, n], f32, name="seT")
        oeT = big.tile([P, shard], f32, name="oeT")
        # exp of projections, transposed: [f, n] / [f, j], split into FB blocks.
        # elT kept in fp32 for the precise normalizer matvec; a bf16 copy
        # (elT16/erT16) feeds the big matmul at full PE streaming rate.
        elT = [big.tile([P, n], f32, name=f"elT{fb}") for fb in range(FB)]
        elTm = ([big.tile([P, n], mm_dt, name=f"elTm{fb}") for fb in range(FB)]
                if use_bf16 else elT)
        erTm = [big.tile([P, shard], mm_dt, name=f"erTm{fb}") for fb in range(FB)]
        vown = const.tile([P, FB], f32, name="vown")
        z_sb = const.tile([P, NT], f32, name="z_sb")
        Z_sb = const.tile([P, NT], f32, name="Z_sb")
        invz = const.tile([P, NT], f32, name="invz")

        # ---- Phase 1: transpose inputs, project, exp, partial normalizer ----
        with tc.tile_pool(name="ps_tr", bufs=3, space="PSUM") as ps_tr, \
             tc.tile_pool(name="ps_pj", bufs=3, space="PSUM") as ps_pj, \
             tc.tile_pool(name="ps_z", bufs=2, space="PSUM") as ps_z:
            for src, dstT, tcnt in ((se_sb, seT, NT), (oe_sb, oeT, JT)):
                for t in range(tcnt):
                    pst = ps_tr.tile([P, P], f32, name="pst", tag="pst")
                    nc.tensor.transpose(pst, src[:, t, :], ident)
                    nc.vector.tensor_copy(out=dstT[:, t * P:(t + 1) * P], in_=pst)

            # LT/RT = projT-block @ (SE.T / OE.T); exp applied on PSUM->SBUF.
            for fb in range(FB):
                pw = proj_sb[:, fb * P:(fb + 1) * P]
                for srcT, dst, width in ((seT, elT[fb], n), (oeT, erTm[fb], shard)):
                    for c0 in range(0, width, NCH):
                        w = min(NCH, width - c0)
                        psl = ps_pj.tile([P, NCH], f32, name="psl", tag="psl")
                        nc.tensor.matmul(psl[:, :w], pw, srcT[:, c0:c0 + w],
                                         start=True, stop=True)
                        nc.scalar.activation(out=dst[:, c0:c0 + w], in_=psl[:, :w],
                                             func=Exp)
                if use_bf16:
                    nc.vector.tensor_copy(out=elTm[fb], in_=elT[fb])

            # v_f = sum over this core's classes of exp(R_jf)
            for fb in range(FB):
                nc.vector.reduce_sum(out=vown[:, fb:fb + 1], in_=erTm[fb], axis=X)

            # Per-row partial normalizer z_i = sum_f exp(L_if) * v_f  -> [n]
            for t in range(NT):
                psz = ps_z.tile([P, 1], f32, name="psz", tag="psz")
                for fb in range(FB):
                    nc.tensor.matmul(psz, elT[fb][:, t * P:(t + 1) * P],
                                     vown[:, fb:fb + 1],
                                     start=(fb == 0), stop=(fb == FB - 1))
                nc.scalar.copy(out=z_sb[:, t:t + 1], in_=psz)

        # ---- AllReduce the per-row partial normalizers (16 KiB) ----
        cc_in = dram.tile([P, NT], f32, name="cc_in")
        cc_out = dram.tile([P, NT], f32, name="cc_out")
        nc.sync.dma_start(out=cc_in, in_=z_sb)
        nc.gpsimd.collective_compute(
            "AllReduce", mybir.AluOpType.add,
            replica_groups=[list(range(n_cores))],
            ins=[cc_in.opt()], outs=[cc_out.opt()],
        )
        nc.sync.dma_start(out=Z_sb, in_=cc_out)
        nc.vector.reciprocal(out=invz, in_=Z_sb)

        # ---- Phase 2: S = exp(L) @ exp(R).T, out = Ln(S * invz), store ----
        with tc.tile_pool(name="ps_mm", bufs=2, space="PSUM") as ps_mm:
            for t in range(NT):
                ob = obp.tile([P, shard], f32, name="ob", tag="ob")
                for jh in range(JH):
                    ps = ps_mm.tile([P, JW], f32, name="ps", tag="ps")
                    for fb in range(FB):
                        lw = elTm[fb][:, t * P:(t + 1) * P]
                        for c0 in range(0, JW, NCH):
                            nc.tensor.matmul(
                                ps[:, c0:c0 + NCH], lw,
                                erTm[fb][:, jh * JW + c0: jh * JW + c0 + NCH],
                                start=(fb == 0), stop=(fb == FB - 1))
                    nc.scalar.activation(out=ob[:, jh * JW:(jh + 1) * JW],
                                         in_=ps, func=Ln,
                                         scale=invz[:, t:t + 1])
                nc.sync.dma_start(out=out_d[t * P:(t + 1) * P, :], in_=ob)

    nc.compile()
    return nc


def _get_nc():
    if "nc" not in _CACHE:
        _CACHE["nc"] = build_bass()
    return _CACHE["nc"]


def _numpy_fallback(start_emb, output_emb, proj, temp):
    L = start_emb.astype(np.float64) @ proj.astype(np.float64)
    R = output_emb.astype(np.float64) @ proj.astype(np.float64)
    mL = L.max(-1, keepdims=True)
    mR = R.max(-1, keepdims=True)
    S = np.exp(L - mL) @ np.exp(R - mR).T
    logits = (np.log(S) + mL + mR.T) / float(temp)
    m = logits.max(-1, keepdims=True)
    out = logits - m - np.log(np.exp(logits - m).sum(-1, keepdims=True))
    return out.astype(np.float32)


def run_on_hw(start_emb, output_emb, proj, trace=False, **trace_kwargs):
    from concourse.bass_utils import run_bass_kernel_spmd

    nc = _get_nc()
    se = np.ascontiguousarray(start_emb, dtype=np.float32)
    oe = np.ascontiguousarray(output_emb, dtype=np.float32)
    pj = np.ascontiguousarray(proj, dtype=np.float32)
    in_maps = [
        {"se": se, "oe": oe[c * SHARD:(c + 1) * SHARD], "proj": pj}
        for c in range(N_CORES)
    ]
    res = run_bass_kernel_spmd(nc, in_maps, core_ids=list(range(N_CORES)),
                               trace=trace, **trace_kwargs)
    out = np.concatenate([res.results[c]["out"] for c in range(N_CORES)], axis=1)
    return out, res


def kernel(start_emb, output_emb, proj, temp):
    t = float(np.asarray(temp).reshape(-1)[0])
    if t != 1.0:
        return _numpy_fallback(np.asarray(start_emb), np.asarray(output_emb),
                               np.asarray(proj), t)
    out, _ = run_on_hw(start_emb, output_emb, proj, trace=False)
    return out
